# revision 35
# baseline (speedup 1.0000x reference)
"""Cross-attention kernel for TRN2, 8 NeuronCores, data-parallel over batch.

Problem (per full input):
    h_enc: [16, 2048, 1024] f32, h_dec: [512, 16, 1024] f32
    e[b,:,:] = h_enc[b] @ h_dec[:,b,:].T          # [T_enc, T_dec]
    a = softmax(e, axis=T_enc)
    c[b] = a.T @ h_enc[b]                         # [T_dec, D]

Sharding: B=16 -> 2 batches per core (embarrassingly parallel, no
collectives). Each core computes its 2 batches; host concatenates.

fp16 on the PE (f32 PSUM accumulation); inputs cast to fp16 on host.

mm1 contracts over d (needs d-major h_enc), mm2 contracts over t_enc
(needs te-major h_enc). Instead of building the d-major copy with PE
tile-transposes (23us of PE time in the old version), the host passes
h_enc in BOTH layouts (heT d-major + he te-major) plus hdT (d-major
h_dec). PE does only the 2 real matmuls (110.6us roofline/core) plus
a few cheap P^T tile-transposes.

Schedule per (batch, t-tile) stage, software-pipelined: slot k emits
[PT(k-1) | mm1(k) | softmax(k) | mm2(k-1)], so softmax(k) (per-bank
DVE maxes -> chunked ACT exp+rowsum -> DVE recip) hides under PE work.
S lives in 4 single-bank PSUM tiles so per-bank deps release early.
P^T runs on the PE (tile-transposes + PSUM->SBUF copies) for batch-0
stages and the final stage, and on the xbar only for mid-batch-1
stages: Tile chains every HWDGE DMA's issue behind all earlier-
scheduled DMAs on its 8 completion lanes, so an xbar PT scheduled
while bulk loads are in flight stalls mm2 by 4-10us (measured).
Batch-1 loads are emitted late (slots (0,3)/(1,0)) and everything
bulky rides the sync ring; the scalar engine/ring stays clear for the
latency-critical exp + PTs. Stage (0,0)'s mm1 is emitted ko(dc)-outer
to stream behind the heT[0] upload; later stages run no-outer. The
final store is split across both HWDGE queues; output is stored fp16
and cast back to f32 on host. Warmup matmuls trip the PE HAM clock
gate during the DMA lead-in.

Measured: 162271 ns (prev baseline) -> 145198/144877 ns, rel_l2 1.7e-3.
"""

import numpy as np

import bass_rust
import concourse.bass as bass
import concourse.mybir as mybir
import concourse.tile as tile
from concourse.bass_utils import run_bass_kernel_spmd
from concourse.masks import make_identity

FP16 = mybir.dt.float16
F32 = mybir.dt.float32

B_FULL = 16
N_CORES = 8
B_PER_CORE = B_FULL // N_CORES  # 2
T_ENC = 2048
T_DEC = 512
D = 1024
P = 128
E_CHUNKS = T_ENC // P  # 16
D_CHUNKS = D // P      # 8
T_CHUNKS = T_DEC // P  # 4
N1 = 512               # matmul1 N tile (one PSUM bank)
N2 = 512               # matmul2 N tile (one PSUM bank)


def split_excess_waits(nc, max_waits: int = 1):
    """This toolchain's walrus accepts only ONE sync-wait command per
    instruction (setupSyncWait raises "Too many sync wait commands"), but
    Tile attaches one wait per producing proc. Hoist excess waits onto
    same-engine NOP carriers inserted just before the instruction."""
    for fn in nc.m.functions:
        for blk in fn.blocks:
            insts = list(blk.instructions)
            new_list = []
            changed = False
            for inst in insts:
                si = inst.sync_info
                waits = list(si.on_wait) if si is not None else []
                if len(waits) > max_waits:
                    changed = True
                    for j, w in enumerate(waits[max_waits:]):
                        nop = mybir.InstNoOp(
                            name=f"{inst.name}-wc{j}",
                            engine=inst.engine,
                            bass_nofuse=True,
                            sync_info=mybir.SyncInfo(on_wait=[w], on_update=[]),
                        )
                        new_list.append(nop)
                    inst.sync_info = bass_rust.SyncInfo(
                        on_wait=waits[:max_waits], on_update=list(si.on_update)
                    )
                new_list.append(inst)
            if changed:
                blk.instructions = new_list


def build_attention_core():
    nc = bass.Bass("TRN2", target_bir_lowering=False, dynamic_dma_scratch_size=1024)
    heT_d = nc.declare_dram_parameter(
        "heT", [B_PER_CORE, D, T_ENC], FP16, isOutput=False
    )
    he_d = nc.declare_dram_parameter(
        "he", [B_PER_CORE, T_ENC, D], FP16, isOutput=False
    )
    hdT_d = nc.declare_dram_parameter(
        "hdT", [B_PER_CORE, D, T_DEC], FP16, isOutput=False
    )
    out = nc.declare_dram_parameter(
        "out", [B_PER_CORE, T_DEC, D], FP16, isOutput=True
    )

    with tile.TileContext(nc) as tc:
        with (
            tc.tile_pool(name="singles", bufs=1) as singles_pool,
            tc.tile_pool(name="p", bufs=2) as p_pool,
            tc.tile_pool(name="pt", bufs=2) as pt_pool,
            tc.tile_pool(name="c", bufs=2) as c_pool,
            tc.tile_pool(name="stats", bufs=4) as stats_pool,
            tc.tile_pool(name="psum_s", bufs=1, space="PSUM") as psum_s_pool,
            tc.tile_pool(name="psum_c", bufs=2, space="PSUM") as psum_c_pool,
            tc.tile_pool(name="psum_t", bufs=2, space="PSUM") as psum_t_pool,
        ):
            # dedicated per-batch input tiles
            # heT[p=d_low, dc, te];  he_nat[p=te_low, ec, d];  hdT[p=d_low, dc, td]
            heT = [
                singles_pool.tile([P, D_CHUNKS, T_ENC], FP16, name=f"heT{b}")
                for b in range(B_PER_CORE)
            ]
            he_nat = [
                singles_pool.tile([P, E_CHUNKS, D], FP16, name=f"he_nat{b}")
                for b in range(B_PER_CORE)
            ]
            hdT = [
                singles_pool.tile([P, D_CHUNKS, T_DEC], FP16, name=f"hdT{b}")
                for b in range(B_PER_CORE)
            ]
            # zeroed dummy for HAM warmup matmuls (results are discarded
            # into the s_psum tag, which the first real mm1 overwrites
            # with start=True)
            dummy = singles_pool.tile([P, P], FP16, name="warm_dummy")
            nc.gpsimd.memset(dummy, 0.0)
            identity = singles_pool.tile([P, P], FP16)
            make_identity(nc, identity)
            # preload the Exp ACT table off the critical path (the first
            # real exp otherwise pays a ~1.3us ACT_TABLE_LOAD)
            tbl_scr = stats_pool.tile([P, 1], F32, tag="tbl_scr")
            nc.scalar.activation(
                out=tbl_scr, in_=dummy[:, :1],
                func=mybir.ActivationFunctionType.Exp, scale=1.0,
            )

            # 64 warmup matmuls: trips the PE HAM clock gate AND bridges
            # the idle window until the first heT chunk lands (~15us), so
            # the streaming mm1(0,0) chunks run at 2.4GHz instead of
            # re-throttled 1.2GHz (measured ~334ns/MM cold vs 216 warm).
            warm_psum = psum_s_pool.tile([P, N1], F32, tag="s0")
            for _ in range(64):
                nc.tensor.matmul(
                    warm_psum[:, :P], lhsT=dummy, rhs=dummy,
                    start=True, stop=True,
                )

            # ---- input loads ----
            def hdT_load(b, dc, eng):
                """one d-chunk of hdT: [128, 512] fp16 (128KB)"""
                eng.dma_start(
                    out=hdT[b][:, dc : dc + 1, :],
                    in_=hdT_d.ap()[b, dc * P : (dc + 1) * P, :].rearrange(
                        "(c p) t -> p c t", p=P
                    ),
                )

            def heT_load(b, dc, eng):
                """one d-chunk (128 rows of heT): [128, 2048] fp16, 4KB lines"""
                eng.dma_start(
                    out=heT[b][:, dc : dc + 1, :],
                    in_=heT_d.ap()[b, dc * P : (dc + 1) * P, :].rearrange(
                        "(c p) t -> p c t", p=P
                    ),
                )

            def he_load(b, sb, half, eng):
                """256 te rows x 512 d cols of he_nat (0.5MB fp16, 1KB lines)"""
                d_sl = slice(half * N2, (half + 1) * N2)
                eng.dma_start(
                    out=he_nat[b][:, 2 * sb : 2 * sb + 2, d_sl],
                    in_=he_d.ap()[b, sb * 256 : (sb + 1) * 256, d_sl].rearrange(
                        "(c p) d -> p c d", p=P
                    ),
                )

            # batch 0 hdT/heT upfront, alternating (hdT_dc, heT_dc) pairs
            # across both HWDGE queues so mm1(0,0) can stream ko-chunks as
            # they land. EVERYTHING else (he_nat, batch 1, stores) rides
            # the sync ring: the scalar engine FIFO must stay clear for
            # the per-stage exp + PT, which gate the PE (ring-throttled
            # DMA issue instructions otherwise block the exp for ~4us).
            # he_nat half 0 first so mm2(0,0) no=0 starts before half 1.
            # measured ring rates: scalar ~146 GB/s, sync ~207 GB/s
            # during the load phase -> 3:5 split of the critical chunks
            for dc in range(D_CHUNKS):
                eng = nc.scalar if dc in (0, 3, 6) else nc.sync
                hdT_load(0, dc, eng)
                heT_load(0, dc, eng)
            for half in range(2):
                for sb in range(T_ENC // 256):
                    he_load(0, sb, half, nc.sync)

            def emit_b1_loads(step):
                # Emitted late (slots (0,3)/(1,0)) so the data lands just
                # in time; the early stages' PTs avoid the DMA completion
                # lane chain by running on the PE instead (see emit_pt_pe).
                if step == 0:
                    for dc in range(D_CHUNKS):
                        hdT_load(1, dc, nc.sync)
                        heT_load(1, dc, nc.sync)
                elif step == 1:
                    for sb in range(T_ENC // 256):
                        he_load(1, sb, 0, nc.sync)
                    for sb in range(T_ENC // 256):
                        he_load(1, sb, 1, nc.sync)

            def emit_pt(stage):
                """P^T via one merged xbar transpose -> pt[p=te_low, ec, td].
                Only used for stages whose DMA-completion lane chain is
                clean by issue time (Tile serializes each DMA's issue
                behind every earlier-scheduled DMA on its ~8 DMAHW lanes,
                so an xbar PT scheduled after bulk loads stalls until
                those complete). Early stages use emit_pt_pe instead."""
                b, m, p_tile, recip = stage
                pt_tile = pt_pool.tile([P, E_CHUNKS, P], FP16, tag="pt")
                nc.scalar.dma_start(out=pt_tile, in_=p_tile, transpose=True)
                return pt_tile

            def emit_pt_pe(stage):
                """P^T via 16 PE tile-transposes (~0.9us of PE) + two
                PSUM->SBUF copies split across DVE and ACT. No DMA, so
                no lane-chain latency; used while bulk loads are still
                in flight and for the final stage (tail latency)."""
                b, m, p_tile, recip = stage
                pt_tile = pt_pool.tile([P, E_CHUNKS, P], FP16, tag="pt")
                for half in range(2):
                    tp = psum_t_pool.tile([P, D_CHUNKS, P], FP16, tag="tp")
                    for j in range(D_CHUNKS):
                        ec = half * D_CHUNKS + j
                        nc.tensor.transpose(
                            tp[:, j, :],
                            p_tile[:, ec * P : (ec + 1) * P],
                            identity,
                        )
                    sl = slice(half * D_CHUNKS, (half + 1) * D_CHUNKS)
                    if half == 0:
                        nc.vector.tensor_copy(pt_tile[:, sl, :], tp)
                    else:
                        nc.scalar.copy(pt_tile[:, sl, :], tp)
                return pt_tile

            def emit_mm2(stage, pt_tile, final=False):
                b, m, p_tile, recip = stage
                m_sl = slice(m * P, (m + 1) * P)
                for no in range(D // N2):
                    c_psum = psum_c_pool.tile([P, N2], F32, tag="c_psum")
                    for ko in range(E_CHUNKS):
                        nc.tensor.matmul(
                            c_psum,
                            lhsT=pt_tile[:, ko, :],
                            rhs=he_nat[b][:, ko, no * N2 : (no + 1) * N2],
                            start=(ko == 0),
                            stop=(ko == E_CHUNKS - 1),
                        )
                    c_sbuf = c_pool.tile([P, N2], FP16, tag="c")
                    nc.vector.tensor_scalar_mul(c_sbuf, c_psum, recip)
                    d_lo = no * N2
                    if final and no == D // N2 - 1:
                        # split the very last store across both HWDGE
                        # queues to halve its drain latency (the tail)
                        h = N2 // 2
                        nc.scalar.dma_start(
                            out=out.ap()[b, m_sl, d_lo : d_lo + h],
                            in_=c_sbuf[:, 0:h],
                        )
                        nc.sync.dma_start(
                            out=out.ap()[b, m_sl, d_lo + h : d_lo + N2],
                            in_=c_sbuf[:, h:N2],
                        )
                    else:
                        nc.sync.dma_start(
                            out=out.ap()[b, m_sl, d_lo : d_lo + N2], in_=c_sbuf
                        )

            prev = None
            for b in range(B_PER_CORE):
                for m in range(T_CHUNKS):
                    if prev is None:
                        pt_prev = None
                    elif prev[0] == 0:
                        pt_prev = emit_pt_pe(prev)
                    else:
                        pt_prev = emit_pt(prev)

                    # ---- matmul1: S = h_dec_tile @ h_enc.T ----
                    # S lives in 4 single-bank PSUM tiles so Tile tracks
                    # per-bank deps: reduces fire as banks stop, and the
                    # next stage's mm1 bank i only waits exp chunk i.
                    # Stage (0,0) runs ko(dc)-outer so each matmul only
                    # needs one uploaded heT d-chunk (streams behind the
                    # DMA); later stages run no-outer (heT resident).
                    s_banks = [
                        psum_s_pool.tile(
                            [P, N1], F32, tag=f"s{no}", name=f"s{no}_{b}_{m}"
                        )
                        for no in range(4)
                    ]
                    if b == 0 and m == 0:
                        for ko in range(D_CHUNKS):
                            for no in range(T_ENC // N1):
                                nc.tensor.matmul(
                                    s_banks[no],
                                    lhsT=hdT[b][:, ko, m * P : (m + 1) * P],
                                    rhs=heT[b][:, ko, no * N1 : (no + 1) * N1],
                                    start=(ko == 0),
                                    stop=(ko == D_CHUNKS - 1),
                                )
                    else:
                        for no in range(T_ENC // N1):
                            for ko in range(D_CHUNKS):
                                nc.tensor.matmul(
                                    s_banks[no],
                                    lhsT=hdT[b][:, ko, m * P : (m + 1) * P],
                                    rhs=heT[b][:, ko, no * N1 : (no + 1) * N1],
                                    start=(ko == 0),
                                    stop=(ko == D_CHUNKS - 1),
                                )

                    if b == 0 and m == 0:
                        # HAM bridge: PE idles ~3.5us here for the heT
                        # DMA tail; these dummies read the already-loaded
                        # dc6 chunk so the scheduler pins them into the
                        # gap, keeping the clock gate warm.
                        warm2 = psum_c_pool.tile([P, N2], F32, tag="c_psum")
                        for _ in range(8):
                            nc.tensor.matmul(
                                warm2[:, :P],
                                lhsT=heT[0][:, 6, 0:P],
                                rhs=heT[0][:, 6, 0:P],
                                start=True, stop=True,
                            )

                    # ---- softmax over free axis (T_enc) ----
                    pmax = stats_pool.tile([P, 4], F32, tag="pmax")
                    for no in range(4):
                        nc.vector.tensor_reduce(
                            out=pmax[:, no : no + 1],
                            in_=s_banks[no],
                            axis=mybir.AxisListType.X,
                            op=mybir.AluOpType.max,
                        )
                    negmax = stats_pool.tile([P, 1], F32, tag="negmax")
                    nc.vector.tensor_reduce(
                        out=negmax,
                        in_=pmax,
                        axis=mybir.AxisListType.X,
                        op=mybir.AluOpType.max,
                        negate=True,
                    )
                    p_tile = p_pool.tile([P, T_ENC], FP16, tag="p")
                    rs4 = stats_pool.tile([P, 4], F32, tag="rs4")
                    for no in range(4):
                        nc.scalar.activation(
                            out=p_tile[:, no * N1 : (no + 1) * N1],
                            in_=s_banks[no],
                            func=mybir.ActivationFunctionType.Exp,
                            bias=negmax,
                            scale=1.0,
                            accum_out=rs4[:, no : no + 1],
                        )
                    rowsum = stats_pool.tile([P, 1], F32, tag="rowsum")
                    nc.vector.tensor_reduce(
                        out=rowsum,
                        in_=rs4,
                        axis=mybir.AxisListType.X,
                        op=mybir.AluOpType.add,
                    )
                    recip = stats_pool.tile([P, 1], F32, tag="recip")
                    nc.vector.reciprocal(recip, rowsum)

                    if (b, m) == (0, 3):
                        emit_b1_loads(0)
                    elif (b, m) == (1, 0):
                        emit_b1_loads(1)

                    # ---- finish the previous stage ----
                    if prev is not None:
                        emit_mm2(prev, pt_prev)
                    prev = (b, m, p_tile, recip)

            # Final stage: P^T on the PE (the xbar PT's ~4us issue->data
            # latency is exposed at the tail with nothing to hide it).
            emit_mm2(prev, emit_pt_pe(prev), final=True)

    split_excess_waits(nc)
    return nc


_NC_CACHE = None


def _get_nc():
    global _NC_CACHE
    if _NC_CACHE is None:
        _NC_CACHE = build_attention_core()
    return _NC_CACHE


def _make_in_maps(h_enc: np.ndarray, h_dec: np.ndarray):
    h_enc16 = h_enc.astype(np.float16)
    h_dec16 = h_dec.astype(np.float16)
    in_maps = []
    for i in range(N_CORES):
        sl = slice(i * B_PER_CORE, (i + 1) * B_PER_CORE)
        he = h_enc16[sl]                              # [2, T_ENC, D]
        hd = h_dec16[:, sl, :]                        # [T_DEC, 2, D]
        in_maps.append(
            {
                "he": np.ascontiguousarray(he),
                "heT": np.ascontiguousarray(he.transpose(0, 2, 1)),
                "hdT": np.ascontiguousarray(hd.transpose(1, 2, 0)),
            }
        )
    return in_maps


def kernel(**inputs) -> np.ndarray:
    h_enc = np.asarray(inputs["h_enc"])
    h_dec = np.asarray(inputs["h_dec"])
    assert h_enc.shape == (B_FULL, T_ENC, D)
    assert h_dec.shape == (T_DEC, B_FULL, D)

    nc = _get_nc()
    in_maps = _make_in_maps(h_enc, h_dec)
    res = run_bass_kernel_spmd(nc, in_maps, core_ids=list(range(N_CORES)))
    out = np.concatenate([res.results[i]["out"] for i in range(N_CORES)], axis=0)
    return np.ascontiguousarray(out.astype(np.float32))


# revision 37
# speedup vs baseline: 1.0039x; 1.0039x over previous
"""Cross-attention kernel for TRN2, 8 NeuronCores, data-parallel over batch.

Problem (per full input):
    h_enc: [16, 2048, 1024] f32, h_dec: [512, 16, 1024] f32
    e[b,:,:] = h_enc[b] @ h_dec[:,b,:].T          # [T_enc, T_dec]
    a = softmax(e, axis=T_enc)
    c[b] = a.T @ h_enc[b]                         # [T_dec, D]

Sharding: B=16 -> 2 batches per core (embarrassingly parallel, no
collectives). Each core computes its 2 batches; host concatenates.

fp16 on the PE (f32 PSUM accumulation); inputs cast to fp16 on host.

mm1 contracts over d (needs d-major h_enc), mm2 contracts over t_enc
(needs te-major h_enc). Instead of building the d-major copy with PE
tile-transposes (23us of PE time in the old version), the host passes
h_enc in BOTH layouts (heT d-major + he te-major) plus hdT (d-major
h_dec). PE does only the 2 real matmuls (110.6us roofline/core) plus
a few cheap P^T tile-transposes.

Schedule per (batch, t-tile) stage, software-pipelined: slot k emits
[PT(k-1) | mm1(k) | softmax(k) | mm2(k-1)], so softmax(k) (per-bank
DVE maxes -> chunked ACT exp+rowsum -> DVE recip) hides under PE work.
S lives in 4 single-bank PSUM tiles so per-bank deps release early.
P^T runs on the PE (tile-transposes + PSUM->SBUF copies) for batch-0
stages and the final stage, and on the xbar only for mid-batch-1
stages: Tile chains every HWDGE DMA's issue behind all earlier-
scheduled DMAs on its 8 completion lanes, so an xbar PT scheduled
while bulk loads are in flight stalls mm2 by 4-10us (measured).
Batch-1 loads are emitted late (slots (0,3)/(1,0)) and everything
bulky rides the sync ring; the scalar engine/ring stays clear for the
latency-critical exp + PTs. Stage (0,0)'s mm1 is emitted ko(dc)-outer
to stream behind the heT[0] upload; later stages run no-outer. The
final store is split across both HWDGE queues; output is stored fp16
and cast back to f32 on host. Warmup matmuls trip the PE HAM clock
gate during the DMA lead-in.

Measured: 162271 ns (prev baseline) -> 145198/144877 ns, rel_l2 1.7e-3.
"""

import numpy as np

import bass_rust
import concourse.bass as bass
import concourse.mybir as mybir
import concourse.tile as tile
from concourse.bass_utils import run_bass_kernel_spmd
from concourse.masks import make_identity

FP16 = mybir.dt.float16
F32 = mybir.dt.float32

B_FULL = 16
N_CORES = 8
B_PER_CORE = B_FULL // N_CORES  # 2
T_ENC = 2048
T_DEC = 512
D = 1024
P = 128
E_CHUNKS = T_ENC // P  # 16
D_CHUNKS = D // P      # 8
T_CHUNKS = T_DEC // P  # 4
N1 = 512               # matmul1 N tile (one PSUM bank)
N2 = 512               # matmul2 N tile (one PSUM bank)


def split_excess_waits(nc, max_waits: int = 1):
    """This toolchain's walrus accepts only ONE sync-wait command per
    instruction (setupSyncWait raises "Too many sync wait commands"), but
    Tile attaches one wait per producing proc. Hoist excess waits onto
    same-engine NOP carriers inserted just before the instruction."""
    for fn in nc.m.functions:
        for blk in fn.blocks:
            insts = list(blk.instructions)
            new_list = []
            changed = False
            for inst in insts:
                si = inst.sync_info
                waits = list(si.on_wait) if si is not None else []
                if len(waits) > max_waits:
                    changed = True
                    for j, w in enumerate(waits[max_waits:]):
                        nop = mybir.InstNoOp(
                            name=f"{inst.name}-wc{j}",
                            engine=inst.engine,
                            bass_nofuse=True,
                            sync_info=mybir.SyncInfo(on_wait=[w], on_update=[]),
                        )
                        new_list.append(nop)
                    inst.sync_info = bass_rust.SyncInfo(
                        on_wait=waits[:max_waits], on_update=list(si.on_update)
                    )
                new_list.append(inst)
            if changed:
                blk.instructions = new_list


def build_attention_core():
    nc = bass.Bass("TRN2", target_bir_lowering=False, dynamic_dma_scratch_size=1024)
    heT_d = nc.declare_dram_parameter(
        "heT", [B_PER_CORE, D, T_ENC], FP16, isOutput=False
    )
    he_d = nc.declare_dram_parameter(
        "he", [B_PER_CORE, T_ENC, D], FP16, isOutput=False
    )
    hdT_d = nc.declare_dram_parameter(
        "hdT", [B_PER_CORE, D, T_DEC], FP16, isOutput=False
    )
    out = nc.declare_dram_parameter(
        "out", [B_PER_CORE, T_DEC, D], FP16, isOutput=True
    )

    with tile.TileContext(nc) as tc:
        with (
            tc.tile_pool(name="singles", bufs=1) as singles_pool,
            tc.tile_pool(name="p", bufs=2) as p_pool,
            tc.tile_pool(name="pt", bufs=2) as pt_pool,
            tc.tile_pool(name="c", bufs=2) as c_pool,
            tc.tile_pool(name="stats", bufs=4) as stats_pool,
            tc.tile_pool(name="psum_s", bufs=1, space="PSUM") as psum_s_pool,
            tc.tile_pool(name="psum_c", bufs=2, space="PSUM") as psum_c_pool,
            tc.tile_pool(name="psum_t", bufs=2, space="PSUM") as psum_t_pool,
        ):
            # dedicated per-batch input tiles
            # heT[p=d_low, dc, te];  he_nat[p=te_low, ec, d];  hdT[p=d_low, dc, td]
            heT = [
                singles_pool.tile([P, D_CHUNKS, T_ENC], FP16, name=f"heT{b}")
                for b in range(B_PER_CORE)
            ]
            he_nat = [
                singles_pool.tile([P, E_CHUNKS, D], FP16, name=f"he_nat{b}")
                for b in range(B_PER_CORE)
            ]
            hdT = [
                singles_pool.tile([P, D_CHUNKS, T_DEC], FP16, name=f"hdT{b}")
                for b in range(B_PER_CORE)
            ]
            # zeroed dummy for HAM warmup matmuls (results are discarded
            # into the s_psum tag, which the first real mm1 overwrites
            # with start=True)
            dummy = singles_pool.tile([P, P], FP16, name="warm_dummy")
            nc.gpsimd.memset(dummy, 0.0)
            identity = singles_pool.tile([P, P], FP16)
            make_identity(nc, identity)
            # preload the Exp ACT table off the critical path (the first
            # real exp otherwise pays a ~1.3us ACT_TABLE_LOAD)
            tbl_scr = stats_pool.tile([P, 1], F32, tag="tbl_scr")
            nc.scalar.activation(
                out=tbl_scr, in_=dummy[:, :1],
                func=mybir.ActivationFunctionType.Exp, scale=1.0,
            )

            warm_psum = psum_s_pool.tile([P, N1], F32, tag="s0")
            for _ in range(40):
                nc.tensor.matmul(
                    warm_psum[:, :P], lhsT=dummy, rhs=dummy,
                    start=True, stop=True,
                )

            # ---- input loads ----
            def hdT_load(b, dc, eng):
                """one d-chunk of hdT: [128, 512] fp16 (128KB)"""
                eng.dma_start(
                    out=hdT[b][:, dc : dc + 1, :],
                    in_=hdT_d.ap()[b, dc * P : (dc + 1) * P, :].rearrange(
                        "(c p) t -> p c t", p=P
                    ),
                )

            def heT_load(b, dc, eng):
                """one d-chunk (128 rows of heT): [128, 2048] fp16, 4KB lines"""
                eng.dma_start(
                    out=heT[b][:, dc : dc + 1, :],
                    in_=heT_d.ap()[b, dc * P : (dc + 1) * P, :].rearrange(
                        "(c p) t -> p c t", p=P
                    ),
                )

            def he_load(b, sb, half, eng):
                """256 te rows x 512 d cols of he_nat (0.5MB fp16, 1KB lines)"""
                d_sl = slice(half * N2, (half + 1) * N2)
                eng.dma_start(
                    out=he_nat[b][:, 2 * sb : 2 * sb + 2, d_sl],
                    in_=he_d.ap()[b, sb * 256 : (sb + 1) * 256, d_sl].rearrange(
                        "(c p) d -> p c d", p=P
                    ),
                )

            # batch 0 hdT/heT upfront, alternating (hdT_dc, heT_dc) pairs
            # across both HWDGE queues so mm1(0,0) can stream ko-chunks as
            # they land. EVERYTHING else (he_nat, batch 1, stores) rides
            # the sync ring: the scalar engine FIFO must stay clear for
            # the per-stage exp + PT, which gate the PE (ring-throttled
            # DMA issue instructions otherwise block the exp for ~4us).
            # he_nat half 0 first so mm2(0,0) no=0 starts before half 1.
            # measured ring rates: scalar ~146 GB/s, sync ~207 GB/s
            # during the load phase -> 3:5 split of the critical chunks
            for dc in range(D_CHUNKS):
                eng = nc.scalar if dc in (0, 3, 6) else nc.sync
                hdT_load(0, dc, eng)
                heT_load(0, dc, eng)
            for half in range(2):
                for sb in range(T_ENC // 256):
                    he_load(0, sb, half, nc.sync)

            def emit_b1_loads(step):
                # Emitted late (slots (0,3)/(1,0)) so the data lands just
                # in time; the early stages' PTs avoid the DMA completion
                # lane chain by running on the PE instead (see emit_pt_pe).
                if step == 0:
                    for dc in range(D_CHUNKS):
                        hdT_load(1, dc, nc.sync)
                        heT_load(1, dc, nc.sync)
                elif step == 1:
                    for sb in range(T_ENC // 256):
                        he_load(1, sb, 0, nc.sync)
                    for sb in range(T_ENC // 256):
                        he_load(1, sb, 1, nc.sync)

            def emit_pt(stage):
                """P^T via one merged xbar transpose -> pt[p=te_low, ec, td].
                Only used for stages whose DMA-completion lane chain is
                clean by issue time (Tile serializes each DMA's issue
                behind every earlier-scheduled DMA on its ~8 DMAHW lanes,
                so an xbar PT scheduled after bulk loads stalls until
                those complete). Early stages use emit_pt_pe instead."""
                b, m, p_tile, recip = stage
                pt_tile = pt_pool.tile([P, E_CHUNKS, P], FP16, tag="pt")
                nc.scalar.dma_start(out=pt_tile, in_=p_tile, transpose=True)
                return pt_tile

            def emit_pt_pe(stage):
                """P^T via 16 PE tile-transposes (~0.9us of PE) + two
                PSUM->SBUF copies split across DVE and ACT. No DMA, so
                no lane-chain latency; used while bulk loads are still
                in flight and for the final stage (tail latency)."""
                b, m, p_tile, recip = stage
                pt_tile = pt_pool.tile([P, E_CHUNKS, P], FP16, tag="pt")
                for half in range(2):
                    tp = psum_t_pool.tile([P, D_CHUNKS, P], FP16, tag="tp")
                    for j in range(D_CHUNKS):
                        ec = half * D_CHUNKS + j
                        nc.tensor.transpose(
                            tp[:, j, :],
                            p_tile[:, ec * P : (ec + 1) * P],
                            identity,
                        )
                    sl = slice(half * D_CHUNKS, (half + 1) * D_CHUNKS)
                    if half == 0:
                        nc.vector.tensor_copy(pt_tile[:, sl, :], tp)
                    else:
                        nc.scalar.copy(pt_tile[:, sl, :], tp)
                return pt_tile

            def emit_mm2(stage, pt_tile, final=False):
                b, m, p_tile, recip = stage
                m_sl = slice(m * P, (m + 1) * P)
                for no in range(D // N2):
                    c_psum = psum_c_pool.tile([P, N2], F32, tag="c_psum")
                    for ko in range(E_CHUNKS):
                        nc.tensor.matmul(
                            c_psum,
                            lhsT=pt_tile[:, ko, :],
                            rhs=he_nat[b][:, ko, no * N2 : (no + 1) * N2],
                            start=(ko == 0),
                            stop=(ko == E_CHUNKS - 1),
                        )
                    c_sbuf = c_pool.tile([P, N2], FP16, tag="c")
                    nc.vector.tensor_scalar_mul(c_sbuf, c_psum, recip)
                    d_lo = no * N2
                    if final and no == D // N2 - 1:
                        # split the very last store across both HWDGE
                        # queues to halve its drain latency (the tail)
                        h = N2 // 2
                        nc.scalar.dma_start(
                            out=out.ap()[b, m_sl, d_lo : d_lo + h],
                            in_=c_sbuf[:, 0:h],
                        )
                        nc.sync.dma_start(
                            out=out.ap()[b, m_sl, d_lo + h : d_lo + N2],
                            in_=c_sbuf[:, h:N2],
                        )
                    else:
                        nc.sync.dma_start(
                            out=out.ap()[b, m_sl, d_lo : d_lo + N2], in_=c_sbuf
                        )

            prev = None
            for b in range(B_PER_CORE):
                for m in range(T_CHUNKS):
                    if prev is None:
                        pt_prev = None
                    elif prev[0] == 0:
                        pt_prev = emit_pt_pe(prev)
                    else:
                        pt_prev = emit_pt(prev)

                    # ---- matmul1: S = h_dec_tile @ h_enc.T ----
                    # S lives in 4 single-bank PSUM tiles so Tile tracks
                    # per-bank deps: reduces fire as banks stop, and the
                    # next stage's mm1 bank i only waits exp chunk i.
                    # Stage (0,0) runs ko(dc)-outer so each matmul only
                    # needs one uploaded heT d-chunk (streams behind the
                    # DMA); later stages run no-outer (heT resident).
                    s_banks = [
                        psum_s_pool.tile(
                            [P, N1], F32, tag=f"s{no}", name=f"s{no}_{b}_{m}"
                        )
                        for no in range(4)
                    ]
                    if b == 0 and m == 0:
                        for ko in range(D_CHUNKS):
                            for no in range(T_ENC // N1):
                                nc.tensor.matmul(
                                    s_banks[no],
                                    lhsT=hdT[b][:, ko, m * P : (m + 1) * P],
                                    rhs=heT[b][:, ko, no * N1 : (no + 1) * N1],
                                    start=(ko == 0),
                                    stop=(ko == D_CHUNKS - 1),
                                )
                    else:
                        for no in range(T_ENC // N1):
                            for ko in range(D_CHUNKS):
                                nc.tensor.matmul(
                                    s_banks[no],
                                    lhsT=hdT[b][:, ko, m * P : (m + 1) * P],
                                    rhs=heT[b][:, ko, no * N1 : (no + 1) * N1],
                                    start=(ko == 0),
                                    stop=(ko == D_CHUNKS - 1),
                                )

                    # (no HAM-bridge dummies here: a trace showed the
                    # scheduler places them in the PE FIFO ahead of
                    # mm1's later ko-chunks, serializing ~20 matmuls
                    # behind the dc6 load and costing ~4us)

                    # ---- softmax over free axis (T_enc) ----
                    pmax = stats_pool.tile([P, 4], F32, tag="pmax")
                    for no in range(4):
                        nc.vector.tensor_reduce(
                            out=pmax[:, no : no + 1],
                            in_=s_banks[no],
                            axis=mybir.AxisListType.X,
                            op=mybir.AluOpType.max,
                        )
                    negmax = stats_pool.tile([P, 1], F32, tag="negmax")
                    nc.vector.tensor_reduce(
                        out=negmax,
                        in_=pmax,
                        axis=mybir.AxisListType.X,
                        op=mybir.AluOpType.max,
                        negate=True,
                    )
                    p_tile = p_pool.tile([P, T_ENC], FP16, tag="p")
                    rs4 = stats_pool.tile([P, 4], F32, tag="rs4")
                    for no in range(4):
                        nc.scalar.activation(
                            out=p_tile[:, no * N1 : (no + 1) * N1],
                            in_=s_banks[no],
                            func=mybir.ActivationFunctionType.Exp,
                            bias=negmax,
                            scale=1.0,
                            accum_out=rs4[:, no : no + 1],
                        )
                    rowsum = stats_pool.tile([P, 1], F32, tag="rowsum")
                    nc.vector.tensor_reduce(
                        out=rowsum,
                        in_=rs4,
                        axis=mybir.AxisListType.X,
                        op=mybir.AluOpType.add,
                    )
                    recip = stats_pool.tile([P, 1], F32, tag="recip")
                    nc.vector.reciprocal(recip, rowsum)

                    if (b, m) == (0, 3):
                        emit_b1_loads(0)
                    elif (b, m) == (1, 0):
                        emit_b1_loads(1)

                    # ---- finish the previous stage ----
                    if prev is not None:
                        emit_mm2(prev, pt_prev)
                    prev = (b, m, p_tile, recip)

            # Final stage: P^T on the PE (the xbar PT's ~4us issue->data
            # latency is exposed at the tail with nothing to hide it).
            emit_mm2(prev, emit_pt_pe(prev), final=True)

    split_excess_waits(nc)
    return nc


_NC_CACHE = None


def _get_nc():
    global _NC_CACHE
    if _NC_CACHE is None:
        _NC_CACHE = build_attention_core()
    return _NC_CACHE


def _make_in_maps(h_enc: np.ndarray, h_dec: np.ndarray):
    h_enc16 = h_enc.astype(np.float16)
    h_dec16 = h_dec.astype(np.float16)
    in_maps = []
    for i in range(N_CORES):
        sl = slice(i * B_PER_CORE, (i + 1) * B_PER_CORE)
        he = h_enc16[sl]                              # [2, T_ENC, D]
        hd = h_dec16[:, sl, :]                        # [T_DEC, 2, D]
        in_maps.append(
            {
                "he": np.ascontiguousarray(he),
                "heT": np.ascontiguousarray(he.transpose(0, 2, 1)),
                "hdT": np.ascontiguousarray(hd.transpose(1, 2, 0)),
            }
        )
    return in_maps


def kernel(**inputs) -> np.ndarray:
    h_enc = np.asarray(inputs["h_enc"])
    h_dec = np.asarray(inputs["h_dec"])
    assert h_enc.shape == (B_FULL, T_ENC, D)
    assert h_dec.shape == (T_DEC, B_FULL, D)

    nc = _get_nc()
    in_maps = _make_in_maps(h_enc, h_dec)
    res = run_bass_kernel_spmd(nc, in_maps, core_ids=list(range(N_CORES)))
    out = np.concatenate([res.results[i]["out"] for i in range(N_CORES)], axis=0)
    return np.ascontiguousarray(out.astype(np.float32))
